# revision 15
# baseline (speedup 1.0000x reference)
# Trainium2 Bass/Tile kernel for causal GQA attention (dense_transformer).
#
# Reference computation (fp32):
#   Q = x@wq, K = x@wk, V = x@wv  (rotary on Q,K; GQA 32 q heads / 8 kv heads)
#   out = softmax(QK^T/sqrt(64), causal) @ V @ wo
#
# Sharding: tensor-parallel over heads (TP=4: 8 q heads + 2 kv heads per
# core) x data-parallel over batch (DP=2: 2 batches per core) = 8 cores.
# Each core computes a partial [2,1024,2048] output (its heads' wo
# contribution); host sums partials within each DP group.
#
# Device pipeline per core (all matmuls bf16 -> fp32 PSUM):
#   phase 1 (per 128-row q-tile): Q and fused K|V projections (contract
#     D=2048 from host-pretransposed x^T tiles), rotary on Q/K via DVE in
#     natural layout, PE-transpose Q/K to head-major [d, q] layout, V kept
#     natural [k, dv] with an appended ones column (softmax denominator).
#   phase 2 (per q-tile, causal over k-tiles): scoresT = K^T-chunk.T @ Q^T
#     with two heads packed in the PE array via row tiling and up to 2
#     k-chunks batched per PSUM bank, one exp per [128,512] bank on ACT
#     (no max subtraction - scores are bounded ~6), diagonal-block causal
#     mask via DVE multiply, AV accumulation for both heads of a pair into
#     one PSUM bank (P^T stationary), normalize by the ones-column sum,
#     PE-transpose to [d, q], W_O.
import numpy as np
import ml_dtypes

B, S, D = 4, 1024, 2048
NH, NKV, HD = 32, 8, 64
TP, DP = 4, 2
QH = NH // TP            # 8 q heads per core
KVH = NKV // TP          # 2 kv heads per core
BL = B // DP             # 2 batches per core
NT = S // 128            # 8 s-tiles per batch
NQT = BL * NT            # 16 q-tiles per core
DC = D // 128            # 16 contraction chunks for the projections
PAIRS = QH // 2          # 4 head pairs (h, h+4) packed per 128 partitions
SCALE = 1.0 / float(np.sqrt(HD))
PERM = [0, 4, 1, 5, 2, 6, 3, 7]   # local head order: pair p = (p, p+4)

bf = ml_dtypes.bfloat16

_built = None


def _build():
    from contextlib import ExitStack
    import concourse.bacc as bacc
    import concourse.tile as tile
    from concourse import mybir

    f32 = mybir.dt.float32
    b16 = mybir.dt.bfloat16
    Exp = mybir.ActivationFunctionType.Exp

    nc = bacc.Bacc("TRN2", target_bir_lowering=False, debug=False,
                   num_devices=TP * DP)

    xt_d = nc.dram_tensor("xt", [NQT, 128, DC, 128], b16, kind="ExternalInput").ap()
    wq_d = nc.dram_tensor("wqr", [DC, 128, QH * HD], b16, kind="ExternalInput").ap()
    wkv_d = nc.dram_tensor("wkvr", [DC, 128, 2 * KVH * HD], b16, kind="ExternalInput").ap()
    wo_d = nc.dram_tensor("wor", [128, PAIRS, D], b16, kind="ExternalInput").ap()
    cos_d = nc.dram_tensor("cosr", [NT, 128, QH * HD // 2], f32, kind="ExternalInput").ap()
    sin_d = nc.dram_tensor("sinr", [NT, 128, QH * HD // 2], f32, kind="ExternalInput").ap()
    mask_d = nc.dram_tensor("maskr", [128, 128], b16, kind="ExternalInput").ap()
    id_d = nc.dram_tensor("identr", [128, 128], b16, kind="ExternalInput").ap()
    y_d = nc.dram_tensor("y", [NQT, 128, D], f32, kind="ExternalOutput").ap()

    with tile.TileContext(nc) as tc:
        with ExitStack() as ctx:
            singles = ctx.enter_context(tc.tile_pool(name="singles", bufs=1))
            ps = ctx.enter_context(tc.tile_pool(name="ps", bufs=4, space="PSUM"))
            pp = ctx.enter_context(tc.tile_pool(name="pp", bufs=2, space="PSUM"))
            xtp = ctx.enter_context(tc.tile_pool(name="xtp", bufs=3))
            rot = ctx.enter_context(tc.tile_pool(name="rot", bufs=2))
            rtmp = ctx.enter_context(tc.tile_pool(name="rtmp", bufs=3))
            persist = ctx.enter_context(tc.tile_pool(name="persist", bufs=1))
            ptp = ctx.enter_context(tc.tile_pool(name="ptp", bufs=6))
            anp = ctx.enter_context(tc.tile_pool(name="anp", bufs=6))
            atp = ctx.enter_context(tc.tile_pool(name="atp", bufs=8))
            outp = ctx.enter_context(tc.tile_pool(name="outp", bufs=2))

            mask_sb = singles.tile([128, 128], b16)
            nc.gpsimd.dma_start(out=mask_sb, in_=mask_d)
            ident_sb = singles.tile([128, 128], b16)
            nc.gpsimd.dma_start(out=ident_sb, in_=id_d)
            # per-chunk weight tiles so the first projection matmul only
            # waits on a ~0.1MB DMA, not the whole weight table
            wq_c, wkv_c, cos_c, sin_c = [], [], [], []
            qeng = [nc.gpsimd, nc.scalar, nc.gpsimd, nc.scalar]
            for c in range(DC):
                wq1 = singles.tile([128, QH * HD], b16, name=f"wq{c}")
                qeng[c % 4].dma_start(out=wq1, in_=wq_d[c])
                wq_c.append(wq1)
                wkv1 = singles.tile([128, 2 * KVH * HD], b16, name=f"wkv{c}")
                qeng[(c + 1) % 4].dma_start(out=wkv1, in_=wkv_d[c])
                wkv_c.append(wkv1)
                if c < NT:
                    cos1 = singles.tile([128, QH * HD // 2], f32, name=f"cos{c}")
                    qeng[(c + 2) % 4].dma_start(out=cos1, in_=cos_d[c])
                    cos_c.append(cos1)
                    sin1 = singles.tile([128, QH * HD // 2], f32, name=f"sin{c}")
                    qeng[(c + 3) % 4].dma_start(out=sin1, in_=sin_d[c])
                    sin_c.append(sin1)
            wo_sb = singles.tile([128, PAIRS, D], b16)
            nc.gpsimd.dma_start(out=wo_sb, in_=wo_d)

            for bl in range(BL):
                qt_tiles = {}
                kt_tiles = {}
                v_tiles = {}

                def phase1(t, bl=bl, qt_tiles=qt_tiles, kt_tiles=kt_tiles,
                           v_tiles=v_tiles):
                    qt = bl * NT + t
                    # ---------- QKV projection ----------
                    xt_sb = xtp.tile([128, DC, 128], b16, tag="xt", name=f"xt{qt}")
                    nc.sync.dma_start(out=xt_sb, in_=xt_d[qt])
                    q_ps = pp.tile([128, QH * HD], f32, tag="q", name=f"qps{qt}")
                    kv_ps = pp.tile([128, 2 * KVH * HD], f32, tag="kv", name=f"kvps{qt}")
                    for c in range(DC):
                        st, sp = (c == 0), (c == DC - 1)
                        nc.tensor.matmul(q_ps, xt_sb[:, c, :], wq_c[c],
                                         start=st, stop=sp)
                        nc.tensor.matmul(kv_ps, xt_sb[:, c, :], wkv_c[c],
                                         start=st, stop=sp)

                    # ---------- rotary (natural layout, pairs on free dim) ----
                    # stage PSUM->SBUF on ACT first: frees the projection
                    # PSUM banks in ~600ns instead of holding them through
                    # the serial DVE rotary chain
                    qf = rot.tile([128, QH * HD], f32, tag="qf", name=f"qf{qt}")
                    nc.scalar.copy(qf, q_ps)
                    kf = rot.tile([128, KVH * HD], f32, tag="kf", name=f"kf{qt}")
                    nc.scalar.copy(kf, kv_ps[:, 0:KVH * HD])
                    c_sl = cos_c[t]
                    s_sl = sin_c[t]
                    qrot = rot.tile([128, QH * HD], b16, tag="qrot", name=f"qr{qt}")
                    qv = qrot.rearrange("p (n two) -> p two n", two=2)
                    qp = qf.rearrange("p (n two) -> p two n", two=2)
                    t1 = rtmp.tile([128, QH * HD // 2], f32, tag="t1", name=f"t1a{qt}")
                    t2 = rtmp.tile([128, QH * HD // 2], f32, tag="t2", name=f"t2a{qt}")
                    nc.vector.tensor_mul(t1, qp[:, 0, :], c_sl)
                    nc.vector.tensor_mul(t2, qp[:, 1, :], s_sl)
                    nc.vector.tensor_sub(qv[:, 0, :], t1, t2)
                    t3 = rtmp.tile([128, QH * HD // 2], f32, tag="t1", name=f"t1b{qt}")
                    t4 = rtmp.tile([128, QH * HD // 2], f32, tag="t2", name=f"t2b{qt}")
                    nc.vector.tensor_mul(t3, qp[:, 0, :], s_sl)
                    nc.vector.tensor_mul(t4, qp[:, 1, :], c_sl)
                    nc.vector.tensor_add(qv[:, 1, :], t3, t4)

                    ck_sl = cos_c[t][:, 0:KVH * HD // 2]
                    sk_sl = sin_c[t][:, 0:KVH * HD // 2]
                    krot = rot.tile([128, KVH * HD], b16, tag="krot", name=f"kr{qt}")
                    kv_ = krot.rearrange("p (n two) -> p two n", two=2)
                    kp = kf.rearrange("p (n two) -> p two n", two=2)
                    u1 = rtmp.tile([128, KVH * HD // 2], f32, tag="u1", name=f"u1a{qt}")
                    u2 = rtmp.tile([128, KVH * HD // 2], f32, tag="u2", name=f"u2a{qt}")
                    nc.vector.tensor_mul(u1, kp[:, 0, :], ck_sl)
                    nc.vector.tensor_mul(u2, kp[:, 1, :], sk_sl)
                    nc.vector.tensor_sub(kv_[:, 0, :], u1, u2)
                    u3 = rtmp.tile([128, KVH * HD // 2], f32, tag="u1", name=f"u1b{qt}")
                    u4 = rtmp.tile([128, KVH * HD // 2], f32, tag="u2", name=f"u2b{qt}")
                    nc.vector.tensor_mul(u3, kp[:, 0, :], sk_sl)
                    nc.vector.tensor_mul(u4, kp[:, 1, :], ck_sl)
                    nc.vector.tensor_add(kv_[:, 1, :], u3, u4)

                    # ---------- V with ones column per kv head ----------
                    v_sb = persist.tile([128, KVH * (HD + 1)], b16, tag="v",
                                        bufs=NT + 2, name=f"v{qt}")
                    voff = KVH * HD
                    nc.vector.tensor_copy(v_sb[:, 0:HD], kv_ps[:, voff:voff + HD])
                    nc.vector.tensor_copy(v_sb[:, HD + 1:2 * HD + 1],
                                   kv_ps[:, voff + HD:voff + 2 * HD])
                    ones_v = v_sb.rearrange("p (h x) -> p h x", x=HD + 1)[:, :, HD:HD + 1]
                    nc.vector.memset(ones_v, 1.0)
                    v_tiles[t] = v_sb

                    # ---------- PE transposes to [d, q] ----------
                    for j in range(PAIRS):
                        tp_ps = ps.tile([128, 128], b16, tag="ps", name=f"tq{qt}_{j}")
                        nc.tensor.transpose(tp_ps, qrot[:, j * 128:(j + 1) * 128],
                                            ident_sb)
                        qt_t = persist.tile([128, 128], b16, tag="qt",
                                            bufs=PAIRS * NT + 2, name=f"qtt{qt}_{j}")
                        nc.scalar.copy(qt_t, tp_ps)
                        qt_tiles[(t, j)] = qt_t
                    kp_ps = ps.tile([128, 128], b16, tag="ps", name=f"tk{qt}")
                    nc.tensor.transpose(kp_ps, krot, ident_sb)
                    kt_t = persist.tile([128, 128], b16, tag="kt", bufs=NT + 2,
                                        name=f"ktt{qt}")
                    nc.scalar.copy(kt_t, kp_ps)
                    kt_tiles[t] = kt_t

                def phase2(t, bl=bl, qt_tiles=qt_tiles, kt_tiles=kt_tiles,
                           v_tiles=v_tiles):
                    qt = bl * NT + t
                    # k-chunks in groups of 4 per head; the two heads of a
                    # pair run as concurrent row-tiled matmuls into SEPARATE
                    # PSUM banks (same-bank concurrent drains are a fatal
                    # write-port collision).  One wide exp per bank on ACT.
                    attnT = []
                    for p in range(PAIRS):
                        avA = ps.tile([128, HD + 1], f32, tag="ps",
                                      name=f"avA{qt}_{p}")
                        avB = ps.tile([128, HD + 1], f32, tag="ps",
                                      name=f"avB{qt}_{p}")
                        qtt = qt_tiles[(t, p)]
                        nkc = t + 1
                        for g0 in range(0, nkc, 4):
                            gk = min(4, nkc - g0)           # chunks in group
                            scA = ps.tile([128, 512], f32, tag="ps",
                                          name=f"scA{qt}_{p}_{g0}")
                            scB = ps.tile([128, 512], f32, tag="ps",
                                          name=f"scB{qt}_{p}_{g0}")
                            for gi in range(gk):
                                ktt = kt_tiles[g0 + gi]
                                o = gi * 128
                                nc.tensor.matmul(scA[:, o:o + 128],
                                                 ktt[0:64, :], qtt[0:64, :],
                                                 start=True, stop=True)
                                nc.tensor.matmul(scB[:, o:o + 128],
                                                 ktt[64:128, :], qtt[64:128, :],
                                                 start=True, stop=True)
                            ptA = ptp.tile([128, 512], b16, tag="pt",
                                           name=f"ptA{qt}_{p}_{g0}")
                            ptB = ptp.tile([128, 512], b16, tag="pt",
                                           name=f"ptB{qt}_{p}_{g0}")
                            nc.scalar.activation(ptA[:, 0:gk * 128], scA[:, 0:gk * 128],
                                                 Exp, scale=SCALE)
                            nc.scalar.activation(ptB[:, 0:gk * 128], scB[:, 0:gk * 128],
                                                 Exp, scale=SCALE)
                            if g0 + gk == nkc:   # diagonal chunk is last
                                o = (gk - 1) * 128
                                nc.vector.tensor_mul(ptA[:, o:o + 128],
                                                     ptA[:, o:o + 128], mask_sb)
                                nc.vector.tensor_mul(ptB[:, o:o + 128],
                                                     ptB[:, o:o + 128], mask_sb)
                            for gi in range(gk):
                                kc = g0 + gi
                                vt = v_tiles[kc]
                                o = gi * 128
                                nc.tensor.matmul(avA, ptA[:, o:o + 128],
                                                 vt[:, 0:HD + 1],
                                                 start=(kc == 0), stop=(kc == t))
                                nc.tensor.matmul(avB, ptB[:, o:o + 128],
                                                 vt[:, HD + 1:2 * (HD + 1)],
                                                 start=(kc == 0), stop=(kc == t))
                        rA = anp.tile([128, 1], f32, tag="recip", name=f"rA{qt}_{p}")
                        rB = anp.tile([128, 1], f32, tag="recip", name=f"rB{qt}_{p}")
                        nc.vector.reciprocal(rA, avA[:, HD:HD + 1])
                        nc.vector.reciprocal(rB, avB[:, HD:HD + 1])
                        atA = anp.tile([128, HD], b16, tag="attn", name=f"aA{qt}_{p}")
                        atB = anp.tile([128, HD], b16, tag="attn", name=f"aB{qt}_{p}")
                        nc.vector.tensor_scalar_mul(atA, avA[:, 0:HD], rA)
                        nc.vector.tensor_scalar_mul(atB, avB[:, 0:HD], rB)
                        tr_ps = ps.tile([128, 128], b16, tag="ps", name=f"trp{qt}_{p}")
                        nc.tensor.transpose(tr_ps[0:64, :], atA, ident_sb)
                        nc.tensor.transpose(tr_ps[64:128, :], atB, ident_sb)
                        aT = atp.tile([128, 128], b16, tag="att", name=f"aT{qt}_{p}")
                        nc.vector.tensor_copy(aT, tr_ps)
                        attnT.append(aT)

                    # ---------- W_O ----------
                    out_sb = outp.tile([128, D], f32, tag="out", name=f"o{qt}")
                    for n in range(4):
                        wo_ps = ps.tile([128, 512], f32, tag="ps", name=f"wops{qt}_{n}")
                        for p in range(PAIRS):
                            nc.tensor.matmul(wo_ps, attnT[p],
                                             wo_sb[:, p, n * 512:(n + 1) * 512],
                                             start=(p == 0), stop=(p == PAIRS - 1))
                        if n < 2:
                            nc.scalar.copy(out_sb[:, n * 512:(n + 1) * 512], wo_ps)
                        else:
                            nc.vector.tensor_copy(out_sb[:, n * 512:(n + 1) * 512], wo_ps)
                    (nc.gpsimd if t % 2 else nc.sync).dma_start(out=y_d[qt], in_=out_sb)

                # software-pipelined emission: phase2 lags phase1 by one tile
                # so the scheduler always has independent projection work for
                # the PE while attention waits on ACT/DVE.
                for t in range(NT):
                    phase1(t)
                    if t >= 1:
                        phase2(t - 1)
                phase2(NT - 1)

    nc.compile()
    return nc


def _prep_core(x, pos_cos, pos_sin, wq, wk, wv, wo, tp, dp):
    gh = [tp * QH + h for h in PERM]
    qcols = np.concatenate([np.arange(g * HD, (g + 1) * HD) for g in gh])
    wqr = wq[:, qcols].astype(bf).reshape(DC, 128, QH * HD).copy()
    kvc = np.arange(tp * KVH * HD, (tp + 1) * KVH * HD)
    wkv = np.concatenate([wk[:, kvc], wv[:, kvc]], axis=1)   # [D, 256]
    wkvr = wkv.astype(bf).reshape(DC, 128, 2 * KVH * HD).copy()
    wor = (wo[qcols, :].astype(bf)
           .reshape(PAIRS, 128, D).transpose(1, 0, 2).copy())
    xs = x[dp * BL:(dp + 1) * BL]
    xt = (xs.reshape(BL, NT, 128, DC, 128).transpose(0, 1, 4, 3, 2)
          .reshape(NQT, 128, DC, 128).astype(bf))
    cosr = (np.tile(pos_cos, (1, QH)).astype(np.float32)
            .reshape(NT, 128, QH * HD // 2).copy())
    sinr = (np.tile(pos_sin, (1, QH)).astype(np.float32)
            .reshape(NT, 128, QH * HD // 2).copy())
    maskr = np.triu(np.ones((128, 128), np.float32)).astype(bf)
    identr = np.eye(128, dtype=np.float32).astype(bf)
    return {"xt": np.ascontiguousarray(xt), "wqr": wqr, "wkvr": wkvr,
            "wor": wor, "cosr": cosr, "sinr": sinr,
            "maskr": maskr, "identr": identr}


def make_in_maps(x, pos_cos, pos_sin, wq, wk, wv, wo):
    x = np.asarray(x, np.float32)
    pos_cos = np.asarray(pos_cos, np.float32)
    pos_sin = np.asarray(pos_sin, np.float32)
    wq = np.asarray(wq, np.float32)
    wk = np.asarray(wk, np.float32)
    wv = np.asarray(wv, np.float32)
    wo = np.asarray(wo, np.float32)
    return [_prep_core(x, pos_cos, pos_sin, wq, wk, wv, wo, c % TP, c // TP)
            for c in range(TP * DP)]


def gather(results):
    y = np.empty((B, S, D), np.float32)
    for dp in range(DP):
        acc = results[dp * TP]["y"].astype(np.float32).copy()
        for t in range(1, TP):
            acc += results[dp * TP + t]["y"]
        y[dp * BL:(dp + 1) * BL] = acc.reshape(BL, S, D)
    return y


def get_nc():
    global _built
    if _built is None:
        _built = _build()
    return _built


def kernel(x, pos_cos, pos_sin, wq, wk, wv, wo):
    from concourse.bass_utils import run_bass_kernel_spmd
    nc = get_nc()
    in_maps = make_in_maps(x, pos_cos, pos_sin, wq, wk, wv, wo)
    res = run_bass_kernel_spmd(nc, in_maps, list(range(TP * DP)))
    return gather(res.results)
